# revision 1
# baseline (speedup 1.0000x reference)
"""Distributed AttentionAutoEncoder kernel for 8 TRN2 NeuronCores (Bass/Tile).

Reference computation (fp32):
    Q = W_q @ X ; K = W_v @ X ; V = W_k @ X          (d=2048, n=8192)
    S = (Q @ K.T) / sqrt(d) ; Z = softmax(S, -1) ; A = Z @ V

Key reformulation: S = W_q @ G @ W_v.T / sqrt(d) with G = X @ X.T, which
removes the Q/K projections entirely.  G contracts over n, so each core
computes G_loc = X_loc @ X_loc.T from its n-shard and one 16MB AllReduce
produces G.  The (d x d) chain T = G @ (W_v.T/sqrt(d)), S = W_q @ T is
column-sharded (256 cols/core) and re-assembled with a cheap AllGather.
V and A are data-parallel over n.

Precision: the softmax logits are huge (|S| ~ 1e3-1e4 post-scale) and
near-one-hot (min top-2 gap ~1.6), so fp22-truncated single-pass fp32
matmuls flip near-tie rows.  G/T/S therefore use fp16 hi/lo split 3-pass
matmuls (hi*hi + hi*lo + lo*hi, fp32 PSUM accumulation) which measure at
~2e-3 absmax relative error end-to-end.  V/P in fp16 (single pass).
"""

import numpy as np

import concourse.bacc as bacc
import concourse.mybir as mybir
import concourse.tile as tile
from concourse.masks import make_identity

P = 128
FP16 = mybir.dt.float16
FP32 = mybir.dt.float32
AF = mybir.ActivationFunctionType

D_FULL = 2048
N_FULL = 8192
NCORES = 8


def build(D=D_FULL, NL=N_FULL // NCORES, NC=NCORES, stop_after=None,
          nchunk=4, agc=2, mock_coll=False):
    """Build the SPMD Bass program (identical on every core)."""
    JS = D // NC          # j-columns of S owned by this core
    nT = NL // P          # n-tiles per core
    dT = D // P           # d-tiles
    CB = min(512, D)      # column-block width over d
    NBS = min(512, NL)    # column-block width over n
    KB = D // CB          # column blocks of d
    NB = NL // NBS        # column blocks of n
    assert NL % NBS == 0 and D % CB == 0

    nc = bacc.Bacc("TRN2", target_bir_lowering=False, debug=False,
                   num_devices=NC)

    # ------------- I/O -------------
    xt_hi = nc.dram_tensor("xt_hi", [NL, D], FP16, kind="ExternalInput")
    xt_lo = nc.dram_tensor("xt_lo", [NL, D], FP16, kind="ExternalInput")
    xn_hi = nc.dram_tensor("xn_hi", [D, NL], FP16, kind="ExternalInput")
    wkt_hi = nc.dram_tensor("wkt_hi", [dT // 4, dT, P, 4 * P], FP16,
                            kind="ExternalInput")
    wvts_hi = nc.dram_tensor("wvts_hi", [D, JS], FP16, kind="ExternalInput")
    wvts_lo = nc.dram_tensor("wvts_lo", [D, JS], FP16, kind="ExternalInput")
    wqt_b = nc.dram_tensor("wqt_b", [dT, P, dT, 2, P], FP16,
                           kind="ExternalInput")
    a_out = nc.dram_tensor("a_out", [D, NL], FP32, kind="ExternalOutput")

    with tile.TileContext(nc) as tc:
        with tc.tile_pool(name="dram", bufs=1, space="DRAM") as dpool:
            NCHUNK = min(nchunk, dT)
            CM = dT // NCHUNK          # m-tiles per AR chunk
            # G is symmetric: chunk c only stores columns >= kb0(c)*CB
            # (blocks at/above the diagonal); the lower triangle is
            # reconstructed by transposing stored tiles in the T phase.
            kb0 = [next(kb for kb in range(KB)
                        if (kb + 1) * CB > c * CM * P)
                   for c in range(NCHUNK)]
            g_in = [dpool.tile([CM * P, (KB - kb0[c]) * CB], FP32,
                               name=f"g_in{c}") for c in range(NCHUNK)]
            _ashr = "Local" if mock_coll else "Shared"
            g_out = [dpool.tile([CM * P, (KB - kb0[c]) * CB], FP32,
                                name=f"g_out{c}",
                                addr_space=_ashr) for c in range(NCHUNK)]
            AGC = min(agc, dT)
            IH = dT // AGC             # i-tiles per AG chunk
            ag_in = [dpool.tile([IH * P, JS], FP32, name=f"ag_in{c}")
                     for c in range(AGC)]
            ag_out = [dpool.tile([NC, IH * P, JS], FP32, name=f"ag_out{c}",
                                 addr_space=_ashr) for c in range(AGC)]
            v_park = dpool.tile([D, NL], FP16, name="v_park")

            # ---------------- Phase 1: G = X X^T (3-pass), then V ----------
            with tc.tile_pool(name="xt", bufs=1) as xt_pool, \
                 tc.tile_pool(name="xn", bufs=1) as xn_pool, \
                 tc.tile_pool(name="vsb", bufs=1) as v_pool:

                xth = []
                xtl = []
                for n in range(nT):
                    th = xt_pool.tile([P, D], FP16, name=f"xth{n}")
                    tl = xt_pool.tile([P, D], FP16, name=f"xtl{n}")
                    nc.sync.dma_start(out=th, in_=xt_hi[n * P:(n + 1) * P, :])
                    nc.sync.dma_start(out=tl, in_=xt_lo[n * P:(n + 1) * P, :])
                    xth.append(th)
                    xtl.append(tl)

                xn_sb = []
                for k in range(dT):
                    t = xn_pool.tile([P, NL], FP16, name=f"xn{k}")
                    nc.sync.dma_start(out=t, in_=xn_hi[k * P:(k + 1) * P, :])
                    xn_sb.append(t)

                # G matmuls: G[m,k] = sum_n XT[n,m] * XT[n,k]
                with tc.tile_pool(name="gstg", bufs=4) as gstg_pool, \
                     tc.tile_pool(name="gps", bufs=4,
                                  space="PSUM") as gps_pool:
                    for m in range(dT):
                        ms = slice(m * P, (m + 1) * P)
                        c = m // CM
                        msl = slice((m % CM) * P, (m % CM + 1) * P)
                        for kb in range(kb0[c], KB):
                            ks = slice(kb * CB, (kb + 1) * CB)
                            ps = gps_pool.tile([P, CB], FP32, name="g_ps",
                                               tag="g_ps")
                            acc = 0
                            last = 3 * nT - 1
                            for n in range(nT):
                                for lh, rh in ((xth[n], xth[n]),
                                               (xth[n], xtl[n]),
                                               (xtl[n], xth[n])):
                                    nc.tensor.matmul(ps, lh[:, ms], rh[:, ks],
                                                     start=(acc == 0),
                                                     stop=(acc == last))
                                    acc += 1
                            stg = gstg_pool.tile([P, CB], FP32, name="g_stg",
                                                 tag="g_stg")
                            nc.scalar.copy(stg, ps)
                            kpk = slice((kb - kb0[c]) * CB,
                                        (kb - kb0[c] + 1) * CB)
                            nc.sync.dma_start(out=g_in[c][msl, kpk], in_=stg)
                        # AllReduce this chunk as soon as its rows are done
                        if m % CM == CM - 1 and stop_after not in ("g",):
                            if mock_coll:
                                nc.sync.dma_start(out=g_out[c][:, :],
                                                  in_=g_in[c][:, :])
                            else:
                                nc.gpsimd.collective_compute(
                                    "AllReduce", mybir.AluOpType.add,
                                    replica_groups=[list(range(NC))],
                                    ins=[g_in[c].opt()], outs=[g_out[c].opt()])

                # V = W_k @ X (single-pass fp16).  k-outer with streamed
                # W_k^T tiles; 4 sweeps of (iv-group x nb).
                v_sb = []
                for iv in range(dT):
                    v_sb.append(v_pool.tile([P, NL], FP16, name=f"v{iv}"))
                # loop: ivg outer; per (ivg, k) load only the 512-col slice
                # of WkT that this iv-group consumes; both nb accumulators
                # live so the slice is read exactly once (8MB total).
                with tc.tile_pool(name="wk", bufs=4) as wk_pool, \
                     tc.tile_pool(name="vps", bufs=4 * NB,
                                  space="PSUM") as vps_pool:
                    for ivg in range(0, dT if stop_after not in ("g", "ar")
                                     else 0, 4):
                        pss = {}
                        for j in range(4):
                            for nb in range(NB):
                                pss[(j, nb)] = vps_pool.tile(
                                    [P, NBS], FP32, name="v_ps", tag="v_ps")
                        for k in range(dT):
                            wt = wk_pool.tile([P, 4 * P], FP16, name="wk_t",
                                              tag="wk_t")
                            nc.sync.dma_start(
                                out=wt, in_=wkt_hi[ivg // 4, k])
                            for j in range(4):
                                for nb in range(NB):
                                    ns = slice(nb * NBS, (nb + 1) * NBS)
                                    nc.tensor.matmul(
                                        pss[(j, nb)],
                                        wt[:, j * P:(j + 1) * P],
                                        xn_sb[k][:, ns],
                                        start=(k == 0), stop=(k == dT - 1))
                        for j in range(4):
                            iv = ivg + j
                            for nb in range(NB):
                                ns = slice(nb * NBS, (nb + 1) * NBS)
                                nc.vector.tensor_copy(out=v_sb[iv][:, ns],
                                                      in_=pss[(j, nb)])
                for iv in range(dT if stop_after not in ("g", "ar") else 0):
                    nc.sync.dma_start(out=v_park[iv * P:(iv + 1) * P, :],
                                      in_=v_sb[iv])

            if stop_after not in ("g", "ar", "v"):
                # ---------------- Phase 2: T = G @ (Wv^T/sqrt(d)) --------------
                th_tiles = []
                tl_tiles = []
                with tc.tile_pool(name="gsb", bufs=1) as g_pool, \
                     tc.tile_pool(name="g32", bufs=2) as g32_pool, \
                     tc.tile_pool(name="wv", bufs=1) as wv_pool, \
                     tc.tile_pool(name="idt", bufs=1) as idt_pool, \
                     tc.tile_pool(name="tsb", bufs=1) as t_pool:

                    wvh = []
                    wvl = []
                    for k in range(dT):
                        h = wv_pool.tile([P, JS], FP16, name=f"wvh{k}")
                        l = wv_pool.tile([P, JS], FP16, name=f"wvl{k}")
                        nc.sync.dma_start(out=h, in_=wvts_hi[k * P:(k + 1) * P, :])
                        nc.sync.dma_start(out=l, in_=wvts_lo[k * P:(k + 1) * P, :])
                        wvh.append(h)
                        wvl.append(l)

                    identT = idt_pool.tile([P, P], FP16, name="identT")
                    make_identity(nc, identT)

                    gh = []
                    gl = []
                    with tc.tile_pool(name="mirps", bufs=4,
                                      space="PSUM") as mir_pool:
                        for k in range(dT):
                            ck = k // CM
                            ksto = slice(kb0[ck] * CB, D)
                            nsto = D - kb0[ck] * CB
                            g32 = g32_pool.tile([P, D], FP32, name="g32",
                                                tag="g32")
                            nc.sync.dma_start(
                                out=g32[:, :nsto],
                                in_=g_out[ck][(k % CM) * P:(k % CM + 1) * P,
                                              :])
                            h = g_pool.tile([P, D], FP16, name=f"gh{k}")
                            l = g_pool.tile([P, D], FP16, name=f"gl{k}")
                            nc.vector.tensor_copy(out=h[:, ksto],
                                                  in_=g32[:, :nsto])
                            nc.vector.tensor_sub(l[:, ksto], g32[:, :nsto],
                                                 h[:, ksto])
                            # mirror below-diagonal tiles: G[k,q] = G[q,k]^T
                            kcs = slice(k * P, (k + 1) * P)
                            for q in range(kb0[ck] * CB // P):
                                assert k * P >= kb0[q // CM] * CB
                                qcs = slice(q * P, (q + 1) * P)
                                for srcl, dst in ((gh, h), (gl, l)):
                                    mp = mir_pool.tile([P, P], FP16,
                                                       name="mir_ps",
                                                       tag="mir_ps")
                                    nc.tensor.transpose(mp, srcl[q][:, kcs],
                                                        identT)
                                    nc.vector.tensor_copy(out=dst[:, qcs],
                                                          in_=mp)
                            gh.append(h)
                            gl.append(l)

                    # T[m,j] = sum_k G[k,m] * WvTs[k,j]  (G symmetric)
                    # k-outer so early matmuls only need the first AR
                    # chunks; m in groups of 8 (PSUM capacity).
                    MG = min(8, dT)
                    tps_ctx = tc.tile_pool(name="tps", bufs=MG, space="PSUM")
                    tps_pool = tps_ctx.__enter__()
                    for mg in range(0, dT, MG):
                        pss = []
                        for m in range(mg, mg + MG):
                            pss.append(tps_pool.tile([P, JS], FP32,
                                                     name="t_ps", tag="t_ps"))
                        last = 3 * dT - 1
                        for k in range(dT):
                            for pi, lh, rh in ((0, gh[k], wvh[k]),
                                               (1, gh[k], wvl[k]),
                                               (2, gl[k], wvh[k])):
                                for j, m in enumerate(range(mg, mg + MG)):
                                    ms = slice(m * P, (m + 1) * P)
                                    nc.tensor.matmul(
                                        pss[j], lh[:, ms], rh,
                                        start=(k == 0 and pi == 0),
                                        stop=(k == dT - 1 and pi == 2))
                        for j, m in enumerate(range(mg, mg + MG)):
                            h = t_pool.tile([P, JS], FP16, name=f"th{m}")
                            l = t_pool.tile([P, JS], FP16, name=f"tl{m}")
                            nc.vector.tensor_copy(out=h, in_=pss[j])
                            nc.vector.tensor_sub(l, pss[j], h)
                            th_tiles.append(h)
                            tl_tiles.append(l)
                    tps_ctx.__exit__(None, None, None)

                    # ------------- Phase 3: S[:, j_c] = W_q @ T ----------------
                    with tc.tile_pool(name="wq", bufs=3) as wq_pool, \
                         tc.tile_pool(name="sstg", bufs=4) as sstg_pool, \
                         tc.tile_pool(name="sps", bufs=4, space="PSUM") as sps_pool:
                        for i in range(dT if stop_after != "t" else 0):
                            ps = sps_pool.tile([P, JS], FP32, name="s_ps",
                                               tag="s_ps")
                            # all (mt, hi/lo) 128x128 blocks for this i in
                            # one contiguous DMA
                            wq_i = wq_pool.tile([P, dT, 2, P], FP16,
                                                name="wq_i", tag="wq_i")
                            nc.sync.dma_start(out=wq_i, in_=wqt_b[i])
                            acc = 0
                            last = 3 * dT - 1
                            for mt in range(dT):
                                wh = wq_i[:, mt, 0, :]
                                wl = wq_i[:, mt, 1, :]
                                for lh, rh in ((wh, th_tiles[mt]),
                                               (wh, tl_tiles[mt]),
                                               (wl, th_tiles[mt])):
                                    nc.tensor.matmul(ps, lh, rh,
                                                     start=(acc == 0),
                                                     stop=(acc == last))
                                    acc += 1
                            stg = sstg_pool.tile([P, JS], FP32, name="s_stg",
                                                 tag="s_stg")
                            nc.scalar.copy(stg, ps)
                            c = i // IH
                            csl = slice((i % IH) * P, (i % IH + 1) * P)
                            nc.sync.dma_start(out=ag_in[c][csl, :], in_=stg)
                            # AllGather chunk once its i-rows are all written
                            if i % IH == IH - 1 and stop_after not in ("t",
                                                                      "s"):
                                if mock_coll:
                                    for r in range(NC):
                                        nc.sync.dma_start(
                                            out=ag_out[c][r, :, :],
                                            in_=ag_in[c][:, :])
                                else:
                                    nc.gpsimd.collective_compute(
                                        "AllGather", mybir.AluOpType.bypass,
                                        replica_groups=[list(range(NC))],
                                        ins=[ag_in[c].opt()],
                                        outs=[ag_out[c].opt()])

                if stop_after not in ("t", "s", "ag"):
                    # ---------------- Phase 4: softmax + A = P @ V ------------------
                    with tc.tile_pool(name="psb", bufs=1) as p_pool, \
                         tc.tile_pool(name="v2", bufs=1) as v2_pool, \
                         tc.tile_pool(name="srow", bufs=2) as srow_pool, \
                         tc.tile_pool(name="stats", bufs=1) as stats_pool, \
                         tc.tile_pool(name="ptsb", bufs=4) as pt_pool, \
                         tc.tile_pool(name="asb", bufs=2) as a_pool, \
                         tc.tile_pool(name="ident", bufs=1) as id_pool, \
                         tc.tile_pool(name="aps", bufs=2, space="PSUM") as aps_pool, \
                         tc.tile_pool(name="ptps", bufs=3, space="PSUM") as ptps_pool:

                        v2_sb = []
                        for iv in range(dT):
                            t = v2_pool.tile([P, NL], FP16, name=f"v2_{iv}")
                            nc.sync.dma_start(out=t,
                                              in_=v_park[iv * P:(iv + 1) * P, :])
                            v2_sb.append(t)

                        ident = id_pool.tile([P, P], FP16, name="ident")
                        make_identity(nc, ident)

                        mx = stats_pool.tile([P, dT], FP32, name="mx")
                        negm = stats_pool.tile([P, dT], FP32, name="negm")
                        ssum = stats_pool.tile([P, dT], FP32, name="ssum")
                        recip = stats_pool.tile([P, dT], FP32, name="recip")

                        p_sb = []
                        for i in range(dT):
                            srow = srow_pool.tile([P, NC, JS], FP32, name="srow",
                                                  tag="srow")
                            # one contiguous 2D DMA per rank block
                            c = i // IH
                            csl = slice((i % IH) * P, (i % IH + 1) * P)
                            for r in range(NC):
                                nc.sync.dma_start(
                                    out=srow[:, r, :],
                                    in_=ag_out[c][r, csl, :])
                            s2d = srow.rearrange("p r j -> p (r j)")
                            nc.vector.reduce_max(mx[:, i:i + 1], s2d,
                                                 axis=mybir.AxisListType.X)
                            nc.scalar.mul(negm[:, i:i + 1], mx[:, i:i + 1], -1.0)
                            pt = p_pool.tile([P, D], FP16, name=f"p{i}")
                            nc.scalar.activation(pt, s2d, AF.Exp,
                                                 bias=negm[:, i:i + 1], scale=1.0,
                                                 accum_out=ssum[:, i:i + 1])
                            nc.vector.reciprocal(recip[:, i:i + 1], ssum[:, i:i + 1])
                            p_sb.append(pt)

                        # A[i,:] = sum_j P[i,j] V[j,:]  (transpose P tiles on PE)
                        for i in range(dT):
                            aps = aps_pool.tile([P, NL], FP32, name="a_ps", tag="a_ps")
                            for jt in range(dT):
                                ptp = ptps_pool.tile([P, P], FP16, name="pt_ps",
                                                     tag="pt_ps")
                                nc.tensor.transpose(ptp,
                                                    p_sb[i][:, jt * P:(jt + 1) * P],
                                                    ident)
                                pts = pt_pool.tile([P, P], FP16, name="pt_sb",
                                                   tag="pt_sb")
                                nc.vector.tensor_copy(out=pts, in_=ptp)
                                for nb in range(NB):
                                    ns = slice(nb * NBS, (nb + 1) * NBS)
                                    nc.tensor.matmul(aps[:, ns], pts,
                                                     v2_sb[jt][:, ns],
                                                     start=(jt == 0),
                                                     stop=(jt == dT - 1))
                            asb = a_pool.tile([P, NL], FP32, name="a_sb", tag="a_sb")
                            nc.vector.tensor_scalar_mul(asb, aps,
                                                        recip[:, i:i + 1])
                            nc.sync.dma_start(out=a_out[i * P:(i + 1) * P, :],
                                              in_=asb)

    nc.compile()
    return nc


def prepare_inputs(X_t, W_q, W_k, W_v, NC=NCORES):
    """Host-side sharding + fp16 hi/lo splits.  Returns in_maps for SPMD."""
    D, N = X_t.shape
    NL = N // NC
    JS = D // NC
    sc = np.float32(1.0) / np.sqrt(np.float32(D))

    def split(a):
        hi = a.astype(np.float16)
        lo = (a.astype(np.float32) - hi.astype(np.float32)).astype(np.float16)
        return np.ascontiguousarray(hi), np.ascontiguousarray(lo)

    dT = D // 128
    P_ = 128

    def block_qt(a):      # [D, D] -> [mt, i, p, q] 128x128 blocks
        return np.ascontiguousarray(
            a.reshape(dT, P_, dT, P_).transpose(0, 2, 1, 3))

    wqt = np.ascontiguousarray(W_q.T.astype(np.float32))
    wqt_hi, wqt_lo = split(wqt)
    # [i, p, mt, h, q]: per-i 1MB contiguous block of all (mt, hi/lo) tiles
    hi4 = wqt_hi.reshape(dT, P_, dT, P_)     # [mt, p, i, q]
    lo4 = wqt_lo.reshape(dT, P_, dT, P_)
    pair = np.stack([hi4, lo4], axis=0)      # [h, mt, p, i, q]
    wqt_b = np.ascontiguousarray(pair.transpose(3, 2, 1, 0, 4))
    wkt_hi = np.ascontiguousarray(W_k.T.astype(np.float16))
    # [D, D] -> [g, k, p, 512] column-group blocks
    wkt_hi = np.ascontiguousarray(
        wkt_hi.reshape(dT, P_, dT // 4, 4 * P_).transpose(2, 0, 1, 3))
    wvts = np.ascontiguousarray((W_v.T.astype(np.float32) * sc))

    in_maps = []
    for c in range(NC):
        xc = np.ascontiguousarray(X_t[:, c * NL:(c + 1) * NL]
                                  .astype(np.float32))
        xt_hi, xt_lo = split(np.ascontiguousarray(xc.T))
        wv_slice = np.ascontiguousarray(wvts[:, c * JS:(c + 1) * JS])
        wvts_hi, wvts_lo = split(wv_slice)
        in_maps.append({
            "xt_hi": xt_hi, "xt_lo": xt_lo,
            "xn_hi": np.ascontiguousarray(xc.astype(np.float16)),
            "wkt_hi": wkt_hi,
            "wvts_hi": wvts_hi, "wvts_lo": wvts_lo,
            "wqt_b": wqt_b,
        })
    return in_maps


_CACHED_NC = None


def _get_nc():
    global _CACHED_NC
    if _CACHED_NC is None:
        _CACHED_NC = build()
    return _CACHED_NC


def run(X_t, W_q, W_k, W_v, trace=False):
    from concourse.bass_utils import run_bass_kernel_spmd
    nc = _get_nc()
    in_maps = prepare_inputs(X_t, W_q, W_k, W_v)
    res = run_bass_kernel_spmd(nc, in_maps, core_ids=list(range(NCORES)),
                               trace=trace)
    A = np.concatenate([res.results[c]["a_out"] for c in range(NCORES)],
                       axis=1)
    return A, res


def kernel(X_t, W_q, W_k, W_v):
    X_t = np.asarray(X_t)
    W_q = np.asarray(W_q)
    W_k = np.asarray(W_k)
    W_v = np.asarray(W_v)
    A, _ = run(X_t, W_q, W_k, W_v, trace=False)
    return A.astype(np.float32)

